# revision 32
# baseline (speedup 1.0000x reference)
"""Trainium2 Bass kernel for the LeNet-C3-style masked conv:
  x [64,6,512,512] f32, W [16,6,5,5] (masked by the C3 connectivity table),
  b [16]  ->  out [64,16,508,508] f32   (VALID conv, stride 1, + bias)

Sharding: data-parallel over batch, 8 images per NeuronCore across 8 cores;
the tiny weights are replicated (pre-arranged host-side into matmul form).

Default scheme ("v2", ~439 us/core in the cost model, tensor-bound at
98.5% PE occupancy; everything bf16 in / fp32 psum / bf16 out):
  - host builds x2[img, ic*2+s, r, w] = x[img, ic, r, w+s] for lanes
    s in {0,1} (w and 2 bottom rows zero-padded), so the kw taps can be
    folded pairwise into the matmul contraction: kw = 2g + s.
  - a psum group covers 16 oc x 6 output rows (M=96); contraction =
    (ic, s, rr in 0..9) = 120 partitions of one [120, 512] bf16 tile.
  - THREE matmuls g=0,1,2 per psum accumulate via rhs column slices
    [2g : 2g+508], with lhsT_g[(ic,s,rr),(oc,dh)] = Weff[oc,ic,rr-dh,2g+s]
    (zero outside 0<=rr-dh<=4 or 2g+s>4).  3 matmuls / 96 rows beats the
    5-tap / 128-row v1 scheme by 1.25x in PE column-cycles; the price is
    2x input bytes (the s-lane) and 10/6 row-overlap reads, which still
    fit under the PE time on the serialized DMA resource (~150 MB/core).
  - evictions with per-partition bias alternate Activation / DVE into
    halves of a [96, 2*508] tile; ONE contiguous 2-dim DMA per psum-pair
    (Pool/SWDGE queue) writes it to a per-pair scratch slot (3-dim
    non-contiguous HBM writes cost ~5x more queue time and were the
    original bottleneck; 2-dim writes keep the queues and the DMA
    resource free for x prefetch).
  - host unshards scratch -> [img,16,508,508] and upcasts bf16 -> f32
    (tolerance is 2e-2; bf16 end-to-end error here measures ~3e-3).
  - warmup=8 + bias_late (the KCFG default): eight 64-col dummy matmuls
    on a DVE-memset tile run during the initial DMA window (t~1.0-1.4us)
    so the PE p-state ramp (full 2.4 GHz only after 3us of continuous
    busy) starts ~1.6us earlier; 439403 -> 438726 ns in the cost model.
    PE is then 98.9% busy with zero mid-kernel idle gaps; the remaining
    ~5us are the head DMA latency (~2.5us: HWDGE-serialized lhsT+x0
    loads + 900ns DMA sem propagation) and the tail (last evict + store
    + drain barriers). Tail findings: Pool SWDGE-store cost scales with
    BYTES (~500ns per [96,508] half, ~783 per pair, ~1567 per quad), so
    batching stores does NOT cut Pool time and only bloats the final
    store; single-half stores (store_halves=1) keep the end-of-kernel
    Pool queue shallow (-283ns), and routing the last two stores via the
    idle SP/HWDGE path (y_tail_spread=2, y_tail_mode="all") dodges the
    Pool queue entirely (-266ns). Splitting the last eviction across
    Act+DVE does NOT help: the Tile framework serializes writers to the
    same output tile. The post-eviction store chain (seq 565 + HWDGE 625
    + dge_dma_delay 650 + transfer + 900ns DMA sem, partly overlapped to
    ~2.1us) and the ~700ns drain-barrier cascade are framework-fixed;
    warmup tiles must have a producer (reads of never-written tiles fail
    tile release), so pe_busy_start can't move before the ~894ns memset
    completion.
Scheme-space notes (measured/derived against the v2 cost model, for
future sessions): matmul cost is ap_size(out free) x 0.4167ns x
cycles_per_row with NO dependence on contraction depth or lhsT columns,
so total PE time = streamed-columns x matmul-count. For 2-lane x2 data
the (6ic x 2s x 10rr | 16oc x 6dh) v2 shape's 0.5 matmuls/output-row is
optimal (any DH=8 variant needs 144 contraction rows > 128). The
0.375/row (2ic x 5s x 12rr | 16oc x 8dh) shape needs 5 shifted x lanes:
as HBM replication that is 126MB/core input -> ~535us on the serialized
360GB/s DMA_ENGINES device (vs 416us total DMA in v2); building lanes
on-chip is also dead (lane blocks are 24 partitions wide, so DVE copies
run at ~1/5 machine width; >2.2us/group vs the 635ns/group budget).
fp8 e4m3 DoubleRow (0.5 cyc/row) fails accuracy: 3.7% rel err measured
vs the 2e-2 gate, and any fp8 residual-correction split lands at >= the
bf16 PE time. The "v1" fallback (KCFG["scheme"]="v1", ~552 us/core,
also HW-verified) is the classic 5-kw-tap scheme over 16-row blocks
described in build_nc()'s body; it needs no x replication.
"""

import numpy as np

import concourse.bass as bass
import concourse.tile as tile
from concourse import bacc, mybir
from concourse.bass_utils import run_bass_kernel_spmd

# ---- problem constants (hardcoded; kernel.py must be self-contained) ----
N_CORES = 8
N_IMG = 64
IMG_PER_CORE = N_IMG // N_CORES
C_IN, C_OUT, KH, KW = 6, 16, 5, 5
H = W = 512
OH = OW = H - KH + 1  # 508
WIN = 20        # input rows per block window
BSTRIDE = 16    # output rows per block
NBLK = 8        # blocks per super-block
SB_STARTS = [0, 128, 256, 380]

# LeNet-5 C3 connectivity: MAP[ic, oc] == 1 iff input channel ic feeds oc.
MAP = np.array([
    [1, 0, 0, 0, 1, 1, 1, 0, 0, 1, 1, 1, 1, 0, 1, 1],
    [1, 1, 0, 0, 0, 1, 1, 1, 0, 0, 1, 1, 1, 1, 0, 1],
    [1, 1, 1, 0, 0, 0, 1, 1, 1, 0, 0, 1, 0, 1, 1, 1],
    [0, 1, 1, 1, 0, 0, 1, 1, 1, 1, 0, 0, 1, 0, 1, 1],
    [0, 0, 1, 1, 1, 0, 0, 1, 1, 1, 1, 0, 1, 1, 0, 1],
    [0, 0, 0, 1, 1, 1, 0, 0, 1, 1, 1, 1, 0, 1, 1, 1],
], dtype=np.float32)  # [in=6, out=16]


def make_lhsT(Weff: np.ndarray) -> np.ndarray:
    """[10, 120, 128]: lhsT[h2*5+kw][(ic*20+rr), (oc*8+dh)]."""
    L = np.zeros((2, KW, C_IN, WIN, C_OUT, 8), dtype=np.float32)
    for h2 in range(2):
        for dh in range(8):
            for kh in range(KH):
                rr = 8 * h2 + dh + kh
                # L[h2, kw, ic, rr, oc, dh] = Weff[oc, ic, kh, kw]
                L[h2, :, :, rr, :, dh] = Weff[:, :, kh, :].transpose(2, 1, 0)
    return L.reshape(10, C_IN * WIN, C_OUT * 8)


# ---- v2 scheme: kw folded pairwise into the contraction dim ----
# Host prepares x2[img, ic*2+s, r, w] = x[img, ic, r, w+s] (s in {0,1},
# zero-padded at w=512 and rows 512..513).  A psum group covers 16 oc x
# DH=6 output rows; contraction = (ic, s, rr in 0..9) = 120 partitions.
# Three matmuls g=0,1,2 accumulate taps kw = 2g+s using rhs column slice
# [2g : 2g+508] of one [120, 512] tile: 3 matmuls per 96x508 psum vs the
# v1 scheme's 5 per 128x508 -> 1.25x less tensor-engine time, paid for
# with 2x input bytes (the s-lane replica) and 10/6 row-overlap reads.
DH2 = 6          # output rows per psum group
RWIN2 = 10       # input rows per window (DH2 + KH - 1)
HPAD2 = H + 2    # x2 plane rows (2 pad rows so the last window reads in-bounds)
NP2 = (OH + DH2 - 1) // DH2  # 85 psum steps per image (last covers 4 rows)


def make_lhsT2(Weff: np.ndarray) -> np.ndarray:
    """[120, 3*96] (SBUF layout): row (ic*2+s)*10+rr, col g*96 + oc*6+dh
    = Weff[oc,ic,rr-dh,2g+s]; contiguous so the load is a plain 2-dim DMA."""
    L = np.zeros((3, C_IN, 2, RWIN2, C_OUT, DH2), dtype=np.float32)
    for g in range(3):
        for s in range(2):
            kw = 2 * g + s
            if kw >= KW:
                continue
            for dh in range(DH2):
                for kh in range(KH):
                    L[g, :, s, dh + kh, :, dh] = Weff[:, :, kh, kw].T
    L = L.reshape(3, C_IN * 2 * RWIN2, C_OUT * DH2)
    return np.ascontiguousarray(L.transpose(1, 0, 2).reshape(120, 3 * 96))


def build_nc_v2(dt: str = "bf16", out_dt: str = "bf16",
                xin_bufs: int = 6, outp_bufs: int = 6, psum_bufs: int = 8,
                in_eng: str = "sync", y_eng: str = "gpsimd",
                evict_engs: tuple = ("scalar", "vector"),
                warmup: int = 0, warmup_ap: int = 64, warmup_memset: int = 512,
                bias_eng: str = "sync", bias_late: bool = False,
                lhsT_eng: str = "sync", last_evict_split: bool = False,
                store_halves: int = 2, y_tail_spread: int = 0,
                y_tail_phase: int = 0, y_tail_mode: str = "alt",
                y_tail_engs: tuple = ("sync",),
                warmup_memset_eng: str = "vector", lhsT_late: bool = False):
    if warmup:
        psum_bufs = min(psum_bufs, 7)  # leave one PSUM bank for the warmup
    ddt = {"f32r": mybir.dt.float32r, "bf16": mybir.dt.bfloat16}[dt]
    ydt = {"f32": mybir.dt.float32, "bf16": mybir.dt.bfloat16}[out_dt]
    nc = bacc.Bacc("TRN2", target_bir_lowering=False, debug=False,
                   num_devices=N_CORES)
    x_h = nc.dram_tensor("x", [IMG_PER_CORE, 2 * C_IN, HPAD2, W], ddt,
                         kind="ExternalInput")
    lhsT_h = nc.dram_tensor("lhsT", [120, 3 * 96], ddt,
                            kind="ExternalInput")
    bias_h = nc.dram_tensor("bias", [96, 1], mybir.dt.float32,
                            kind="ExternalInput")
    # Batch store_halves psum steps per output DMA: each SWDGE desc-gen job
    # costs ~783ns of Pool-engine time regardless of payload, and the last
    # few stores serialize on Pool after the final matmuls -> fewer, bigger
    # stores shrink both the Pool load and the kernel tail.
    H = store_halves
    nst = (NP2 + H - 1) // H  # stores per image (last one may be partial)
    y_h = nc.dram_tensor(
        "y", [IMG_PER_CORE, nst, 96, H * OW], ydt, kind="ExternalOutput")

    with tile.TileContext(nc) as tc:
        with (
            tc.tile_pool(name="consts", bufs=1) as consts,
            tc.tile_pool(name="xin", bufs=xin_bufs) as xin,
            tc.tile_pool(name="outp", bufs=outp_bufs) as outp,
            tc.tile_pool(name="psum", bufs=psum_bufs, space="PSUM") as psum,
        ):
            lhsT_t = consts.tile([120, 3 * 96], ddt)
            if not lhsT_late:
                getattr(nc, lhsT_eng).dma_start(out=lhsT_t[:],
                                                in_=lhsT_h.ap())
            bias_t = consts.tile([96, 1], mybir.dt.float32)
            if not bias_late:
                # bias isn't needed until the first eviction (~3.5us in), so
                # issuing it before the first x tile only delays matmul #1.
                getattr(nc, bias_eng).dma_start(out=bias_t[:],
                                                in_=bias_h.ap())

            if warmup:
                # Dummy matmuls on a memset tile bridge the initial const/x
                # DMA window so the PE p-state clock is fully ramped (3 us of
                # continuous execution) by the time real matmuls start.
                # warmup_ap narrows the dummy rhs so the bridge is
                # fine-grained and doesn't overshoot the first real matmul.
                wt = consts.tile([128, 512], ddt)
                # memset only the columns the warmup matmuls read; shorter
                # memset -> earlier pe_busy_start -> earlier full p-state.
                # warmup_memset=0 skips it entirely: the dummy matmuls then
                # read uninitialized SBUF (their psum is never read, so any
                # garbage/NaN is harmless) and pe_busy_start moves to ~400ns.
                if warmup_memset:
                    getattr(nc, warmup_memset_eng).memset(
                        wt[:, 0:warmup_memset], 0.0)
                with tc.tile_pool(name="wpsum", bufs=1,
                                  space="PSUM") as wpool:
                    wps = wpool.tile([128, warmup_ap], mybir.dt.float32)
                    for i in range(warmup):
                        nc.tensor.matmul(wps[:], wt[:, 0:128],
                                         wt[:, 0:warmup_ap],
                                         start=(i == 0),
                                         stop=(i == warmup - 1))

            for img in range(IMG_PER_CORE):
                ot = None
                for p in range(NP2):
                    sti, slot = divmod(p, H)
                    if slot == 0:
                        ot = outp.tile([96, H * OW], ydt)
                    r0 = DH2 * p
                    xt = xin.tile([120, 512], ddt)
                    getattr(nc, in_eng).dma_start(
                        out=xt[:],
                        in_=bass.AP(
                            tensor=x_h.ap().tensor,
                            offset=img * 2 * C_IN * HPAD2 * W + r0 * W,
                            ap=[[HPAD2 * W, 2 * C_IN], [W, RWIN2],
                                [1, W]],
                        ),
                    )
                    if img == 0 and p == 0:
                        # xt0 was issued first so its (longer) transfer heads
                        # the SP/HWDGE queue; lhsT's shorter transfer follows.
                        if lhsT_late:
                            getattr(nc, lhsT_eng).dma_start(
                                out=lhsT_t[:], in_=lhsT_h.ap())
                        if bias_late:
                            getattr(nc, bias_eng).dma_start(
                                out=bias_t[:], in_=bias_h.ap())
                    ps = psum.tile([96, OW], mybir.dt.float32)
                    for g in range(3):
                        nc.tensor.matmul(
                            ps[:],
                            lhsT_t[:, g * 96:(g + 1) * 96],
                            xt[:, 2 * g: 2 * g + OW],
                            start=(g == 0),
                            stop=(g == 2),
                        )
                    osl = ot[:, slot * OW:(slot + 1) * OW]
                    is_last = (img == IMG_PER_CORE - 1 and p == NP2 - 1)
                    if last_evict_split and is_last:
                        # final eviction is on the critical tail: split it
                        # across both engines so its latency halves.
                        hw_ = OW // 2
                        nc.scalar.activation(
                            osl[:, 0:hw_], ps[:, 0:hw_],
                            mybir.ActivationFunctionType.Identity,
                            bias=bias_t[:],
                        )
                        nc.vector.tensor_scalar_add(
                            osl[:, hw_:OW], ps[:, hw_:OW], bias_t[:])
                    elif evict_engs[p % len(evict_engs)] == "scalar":
                        nc.scalar.activation(
                            osl, ps[:],
                            mybir.ActivationFunctionType.Identity,
                            bias=bias_t[:],
                        )
                    else:
                        nc.vector.tensor_scalar_add(osl, ps[:], bias_t[:])
                    if slot == H - 1 or p == NP2 - 1:
                        # near the very end Pool's store queue serializes the
                        # tail; SP is idle once the last x tile is loaded, so
                        # alternate the last image's final stores onto it.
                        yeng = y_eng
                        if (img == IMG_PER_CORE - 1
                                and p >= NP2 - y_tail_spread
                                and (y_tail_mode == "all"
                                     or (NP2 - 1 - p) % 2 == y_tail_phase)):
                            yeng = y_tail_engs[(NP2 - 1 - p)
                                               % len(y_tail_engs)]
                        nslot = slot + 1
                        if nslot == H:
                            getattr(nc, yeng).dma_start(
                                out=y_h.ap()[img, sti], in_=ot[:])
                        else:
                            getattr(nc, yeng).dma_start(
                                out=bass.AP(tensor=y_h.ap().tensor,
                                            offset=(img * nst + sti)
                                            * 96 * H * OW,
                                            ap=[[H * OW, 96],
                                                [1, nslot * OW]]),
                                in_=ot[:, 0:nslot * OW])
    nc.compile()
    return nc


def prep_x2(shards: np.ndarray) -> np.ndarray:
    """[8, IMG, 6, 512, 512] f32 -> [8, IMG, 12, 514, 512] (lane s shifts w by s)."""
    n_cores, n_img = shards.shape[:2]
    x2 = np.zeros((n_cores, n_img, C_IN, 2, HPAD2, W), dtype=shards.dtype)
    x2[:, :, :, 0, :H, :] = shards
    x2[:, :, :, 1, :H, :W - 1] = shards[..., 1:]
    return np.ascontiguousarray(
        x2.reshape(n_cores, n_img, 2 * C_IN, HPAD2, W))


def unshard_v2(y_sc: np.ndarray) -> np.ndarray:
    """[img, nst, 96, H*508] -> [img, 16, 508, 508] f32."""
    n, nst = y_sc.shape[:2]
    H = y_sc.shape[3] // OW
    arr = np.asarray(y_sc, np.float32).reshape(
        n, nst, C_OUT, DH2, H, OW)
    # rows = (store*H + slot)*6 + dh
    arr = arr.transpose(0, 2, 1, 4, 3, 5).reshape(
        n, C_OUT, nst * H * DH2, OW)
    return np.ascontiguousarray(arr[:, :, :OH, :])


def build_nc(dt: str = "bf16", out_dt: str = "bf16",
             xin_bufs: int = 6, outp_bufs: int = 6, psum_bufs: int = 8,
             in_eng: str = "sync", y_eng: str = "sync",
             evict_engs: tuple = ("scalar", "vector")):
    ddt = {"f32r": mybir.dt.float32r, "bf16": mybir.dt.bfloat16}[dt]
    ydt = {"f32": mybir.dt.float32, "bf16": mybir.dt.bfloat16}[out_dt]
    nc = bacc.Bacc("TRN2", target_bir_lowering=False, debug=False,
                   num_devices=N_CORES)
    x_h = nc.dram_tensor("x", [IMG_PER_CORE, C_IN, H, W], ddt,
                         kind="ExternalInput")
    lhsT_h = nc.dram_tensor("lhsT", [10, 120, 128], ddt,
                            kind="ExternalInput")
    bias_h = nc.dram_tensor("bias", [128, 1], mybir.dt.float32,
                            kind="ExternalInput")
    # per-block contiguous scratch: [img, sb, blk, 128, 2*508]
    y_h = nc.dram_tensor(
        "y", [IMG_PER_CORE, len(SB_STARTS), NBLK, 128, 2 * OW], ydt,
        kind="ExternalOutput")

    with tile.TileContext(nc) as tc:
        with (
            tc.tile_pool(name="consts", bufs=1) as consts,
            tc.tile_pool(name="xin", bufs=xin_bufs) as xin,
            tc.tile_pool(name="outp", bufs=outp_bufs) as outp,
            tc.tile_pool(name="psum", bufs=psum_bufs, space="PSUM") as psum,
        ):
            lhsT_t = consts.tile([120, 10 * 128], ddt)
            nc.sync.dma_start(
                out=lhsT_t[:],
                in_=bass.AP(tensor=lhsT_h.ap().tensor, offset=0,
                            ap=[[128, 120], [120 * 128, 10], [1, 128]]),
            )
            bias_t = consts.tile([128, 1], mybir.dt.float32)
            nc.sync.dma_start(out=bias_t[:], in_=bias_h.ap())

            for img in range(IMG_PER_CORE):
                for sbi, S in enumerate(SB_STARTS):
                    for blk in range(NBLK):
                        r0 = S + blk * BSTRIDE
                        xt = xin.tile([120, 512], ddt)
                        getattr(nc, in_eng).dma_start(
                            out=xt[:],
                            in_=bass.AP(
                                tensor=x_h.ap().tensor,
                                offset=img * C_IN * H * W + r0 * W,
                                ap=[[H * W, C_IN], [W, WIN], [1, W]],
                            ),
                        )
                        ot = outp.tile([128, 2 * OW], ydt)
                        for h2 in range(2):
                            ps = psum.tile([128, OW], mybir.dt.float32)
                            for kw in range(KW):
                                j = h2 * 5 + kw
                                nc.tensor.matmul(
                                    ps[:],
                                    lhsT_t[:, j * 128:(j + 1) * 128],
                                    xt[:, kw: kw + OW],
                                    start=(kw == 0),
                                    stop=(kw == KW - 1),
                                )
                            osl = ot[:, h2 * OW:(h2 + 1) * OW]
                            ev = evict_engs[h2 % len(evict_engs)]
                            if ev == "scalar":
                                nc.scalar.activation(
                                    osl, ps[:],
                                    mybir.ActivationFunctionType.Identity,
                                    bias=bias_t[:],
                                )
                            else:
                                nc.vector.tensor_scalar_add(
                                    osl, ps[:], bias_t[:])
                        getattr(nc, y_eng).dma_start(
                            out=y_h.ap()[img, sbi, blk],
                            in_=ot[:],
                        )
    nc.compile()
    return nc


_NC_CACHE = {}

# default build configuration used by kernel()
# warmup=8: eight 64-col dummy matmuls on a memset tile bridge the initial
# DMA window (t~0.9-1.4us) so the PE p-state ramp (full clock only after 3us
# of continuous busy) starts ~1.6us earlier; bias_late defers the bias DMA
# behind the first x tile; store_halves=1: single-half output stores keep
# the Pool SWDGE queue shallow at the end of the kernel; y_tail_spread=2 +
# y_tail_mode="all": the last two stores go out via the (idle) SP/HWDGE
# path instead of queueing behind Pool. CoreSim: 439403 -> 438177 ns.
KCFG = dict(scheme="v2", warmup=8, bias_late=True, store_halves=1,
            y_tail_spread=2, y_tail_mode="all")


def _get_nc(**kw):
    key = tuple(sorted(kw.items()))
    if key not in _NC_CACHE:
        kw = dict(kw)
        scheme = kw.pop("scheme", "v1")
        fn = {"v1": build_nc, "v2": build_nc_v2}[scheme]
        _NC_CACHE[key] = fn(**kw)
    return _NC_CACHE[key]


def unshard_scratch(y_sc: np.ndarray) -> np.ndarray:
    """[img, 4, 8, 128, 2*508] per-core scratch -> [img,16,508,508] f32."""
    n = y_sc.shape[0]
    arr = np.asarray(y_sc, np.float32).reshape(
        n, len(SB_STARTS), NBLK, C_OUT, 8, 2, OW)
    out = np.empty((n, C_OUT, OH, OW), dtype=np.float32)
    for sbi, S in enumerate(SB_STARTS):
        # rows S + blk*16 + h2*8 + dh  <-  arr[:, sbi, blk, oc, dh, h2, :]
        blkv = arr[:, sbi].transpose(0, 2, 1, 4, 3, 5)  # n, oc, blk, h2, dh, w
        out[:, :, S:S + 128, :] = blkv.reshape(n, C_OUT, 128, OW)
    return out


def _prep_inputs(x: np.ndarray, Wt: np.ndarray, b: np.ndarray,
                 dt: str = "bf16", scheme: str = "v1"):
    Weff = np.asarray(Wt, np.float32) * MAP.T[:, :, None, None]
    shards = np.ascontiguousarray(
        np.asarray(x, np.float32).reshape(N_CORES, IMG_PER_CORE, C_IN, H, W))
    if scheme == "v2":
        lhsT = make_lhsT2(Weff)
        bias = np.repeat(np.asarray(b, np.float32), DH2).reshape(96, 1)
        shards = prep_x2(shards)
    else:
        lhsT = make_lhsT(Weff)
        bias = np.repeat(np.asarray(b, np.float32), 8).reshape(128, 1)
    if dt == "bf16":
        import ml_dtypes
        lhsT = lhsT.astype(ml_dtypes.bfloat16)
        shards = shards.astype(ml_dtypes.bfloat16)
    return [{"x": shards[i], "lhsT": lhsT, "bias": bias}
            for i in range(N_CORES)]


def _run(inputs: dict, **spmd_kwargs):
    nc = _get_nc(**KCFG)
    scheme = KCFG.get("scheme", "v1")
    in_maps = _prep_inputs(inputs["x"], inputs["W"], inputs["b"],
                           dt=KCFG.get("dt", "bf16"), scheme=scheme)
    res = run_bass_kernel_spmd(nc, in_maps, list(range(N_CORES)),
                               **spmd_kwargs)
    unshard = unshard_v2 if scheme == "v2" else unshard_scratch
    y = np.concatenate([unshard(r["y"]) for r in res.results], axis=0)
    return y, res


def kernel(**inputs) -> np.ndarray:
    y, _ = _run(inputs)
    return y

